# revision 12
# baseline (speedup 1.0000x reference)
# Mixture-of-two-experts (modality-routed) token GEMM on 8 Trainium2 NeuronCores.
#
# v4: weights-stationary. The reference computes BOTH expert GEMMs and selects
# per token; only one GEMM per token is needed. Host partitions tokens by
# type_id (expert-dispatch): cores 0-3 carry expert-0 tokens + W0, cores 4-7
# expert-1 tokens + W1 (weights arrive as data, the per-core program is
# identical). On device each core computes y[e, tok] = W x + b with W tiles
# STATIONARY and the token dim MOVING, so PE cost scales with the actual
# per-core token count (n_tok rounded to 16) instead of 128-padded m-tiles.
#
# Mixed precision: contraction k 0..1279 runs fp16; k 1280..2047 runs as three
# fp8e4m3 DoubleRow pairs (2 k-planes per PE cell, 2 MACs/cycle) - 13 PE
# passes over k instead of 16. SW=45.25 places max|W|*SW at 1.0 so the top
# e4m3 binade is [0.5,1) with step 1/16 (SW=64 wastes half the mantissa:
# max lands mid-binade at 1.41 where the step is 1/8). Host-simulated with
# exact device semantics: rel err 1.8555e-2, inside the 2e-2 gate (the b=2
# config's host sim matched hardware to 2e-6).
# Scales: x8 = x*16, w8 = W*45.25, fp16 W pre-scaled *724 so one PSUM chain
# is consistent at 724*y; the host divides the fp16 output by 724.

import os
import sys
import time

import numpy as np
import ml_dtypes

for _p in ("/opt/trn_rl_repo", "/root/.axon_site/_ro/trn_rl_repo"):
    if os.path.isdir(_p) and _p not in sys.path:
        sys.path.insert(0, _p)

import concourse.bacc as bacc
import concourse.mybir as mybir
import concourse.tile as tile
from concourse.bass_utils import run_bass_kernel_spmd

D = 2048
ET = D // 128  # 16 output-feature tiles
K16 = 10  # fp16 k-tiles (k 0..1279)
NPAIR = 3  # fp8 DoubleRow pairs (k 1280..2047, 256 contraction rows each)
KSPLIT = K16 * 128
SX = 16.0  # fp8 scale on x
SW = 45.25  # fp8 scale on W (max|W|*SW = 1.0: top binade step 1/16)
SCALE = SX * SW  # PSUM carries SCALE*y; host divides it out
N_CORES = 8
CORES_PER_EXPERT = 4
N_WARMUP = 1  # PE warm-up matmuls. Every transfer is striped half/half
# across the sync+scalar HWDGE rings, and the first-MM critical set
# (w0[:,0:128] + xh0) leads both rings, so the first real operands land
# ~8.1us - right at PE boot. Excess zero-MMs sit AHEAD of ready real work
# in the PE queue and delay it (~430-790ns each at p-state-ramp clock).
F8 = ml_dtypes.float8_e4m3fn

_PROGRAM_CACHE: dict[int, object] = {}
LAST_RESULTS = None  # BassKernelResults of the most recent launch (for profiling)


def _chunks(n_tok: int):
    """Split the token dim into <=512-wide chunks, 4-aligned boundaries.

    Chunk 0 is full 512 wide when possible: during the DMA ramp the 8
    in-flight chunk-0 chains then expose 8*216ns of PE work per arriving
    (w_k, xh_k) pair, matching the ~1.7us pair cadence."""
    if n_tok <= 512:
        return [(0, n_tok)]
    rest = n_tok - 512
    nch = -(-rest // 512)
    base = rest // nch // 4 * 4
    sizes = [base] * nch
    i = 0
    while 512 + sum(sizes) < n_tok:
        sizes[i] += 4
        i = (i + 1) % nch
    sizes = [512] + sizes
    assert 512 + rest == n_tok and all(s <= 512 for s in sizes)
    out, s0 = [], 0
    for s in sizes:
        out.append((s0, s))
        s0 += s
    return out


def _build_program(n_tok: int):
    """One NeuronCore program: y[e, tok] = SCALE * (W @ x + bias), fp16 out."""
    assert n_tok % 4 == 0
    f16 = mybir.dt.float16
    f32 = mybir.dt.float32
    f8 = mybir.dt.float8e4
    DR = mybir.MatmulPerfMode.DoubleRow

    nc = bacc.Bacc("TRN2", target_bir_lowering=False, debug=False, num_devices=N_CORES)
    xt = nc.dram_tensor("xt", [K16, 128, n_tok], f16, kind="ExternalInput").ap()
    xt8 = nc.dram_tensor("xt8", [NPAIR, 128, 2, n_tok], f8, kind="ExternalInput").ap()
    wt = nc.dram_tensor("wt", [K16, 128, D], f16, kind="ExternalInput").ap()
    wt8 = nc.dram_tensor("wt8", [NPAIR, 128, 2, D], f8, kind="ExternalInput").ap()
    biasw = nc.dram_tensor("biasw", [128, ET], f32, kind="ExternalInput").ap()
    y = nc.dram_tensor("y", [ET, 128, n_tok], f16, kind="ExternalOutput").ap()

    CH = _chunks(n_tok)
    ch0 = CH[0][1]  # chunk-0 width: the x "head" loaded before the tails
    tail = n_tok - ch0

    # k-units: ('f', k) = one fp16 k-tile, ('d', j) = one fp8 DoubleRow pair.
    units = [("f", k) for k in range(K16)] + [("d", j) for j in range(NPAIR)]

    with tile.TileContext(nc) as tc:
        with (
            tc.tile_pool(name="wp", bufs=1) as wp,
            tc.tile_pool(name="xp", bufs=1) as xp,
            tc.tile_pool(name="bp", bufs=1) as bp,
            # ot staging: enough bufs that DVE drains never wait on y-DMA
            # completion (the y DMAs queue up behind the input loads on the
            # sync HWDGE ring until ~50us - with few bufs the DVE stalls on
            # slot reuse, the PE runs dry behind it, and HAM re-throttles)
            tc.tile_pool(name="op", bufs=14) as op_,
            tc.tile_pool(name="pp", bufs=8, space="PSUM") as pp,
        ):
            # (w_k, x-head_k) pairs in unit order, every object striped
            # half/half across BOTH HWDGE rings (sync + scalar). While both
            # rings are backlogged they share the ~360GB/s HBM pool, so the
            # pair cadence equals the single-ring one - but completion is
            # smooth (256KB lumps in lockstep) instead of bursty, and the
            # first-MM critical set (w0[:,0:128] + xh0) leads BOTH rings, so
            # the first chain starts ~2us earlier than one-ring FIFO. FIFO
            # order within each ring still doubles as a priority scheme.
            rings = (nc.sync, nc.scalar)
            bias_s = bp.tile([128, ET], f32, name="bias_s")
            wk, xh = [], []
            for k in range(K16):
                ws = wp.tile([128, D], f16, name=f"w{k}", tag=f"w{k}")
                h = xp.tile([128, ch0], f16, name=f"xh{k}", tag=f"xh{k}")
                if k == 0:
                    # ring0: first e-tile of w0 (32KB) then the rest of the
                    # low half; ring1: the full xh0 head (128KB), bias, w0-hi.
                    # Chain (c0,e0) is ready after just the two lead transfers.
                    nc.sync.dma_start(ws[:, 0:128], wt[k][:, 0:128])
                    nc.scalar.dma_start(h[:], xt[k][:, 0:ch0])
                    nc.sync.dma_start(ws[:, 128 : D // 2], wt[k][:, 128 : D // 2])
                    nc.scalar.dma_start(bias_s[:], biasw[:])
                    nc.scalar.dma_start(ws[:, D // 2 : D], wt[k][:, D // 2 : D])
                else:
                    nc.sync.dma_start(ws[:, 0 : D // 2], wt[k][:, 0 : D // 2])
                    nc.scalar.dma_start(ws[:, D // 2 : D], wt[k][:, D // 2 : D])
                    nc.sync.dma_start(h[:, 0 : ch0 // 2], xt[k][:, 0 : ch0 // 2])
                    nc.scalar.dma_start(h[:, ch0 // 2 : ch0], xt[k][:, ch0 // 2 : ch0])
                wk.append(ws)
                xh.append(h)
            w8t, x8h = [], []
            for j in range(NPAIR):
                w8 = wp.tile([128, 2, D], f8, name=f"w8_{j}", tag=f"w8_{j}")
                nc.sync.dma_start(w8[:, 0, :], wt8[j][:, 0, :])
                nc.scalar.dma_start(w8[:, 1, :], wt8[j][:, 1, :])
                w8t.append(w8)
                h8 = xp.tile([128, 2, ch0], f8, name=f"x8h{j}", tag=f"x8h{j}")
                nc.sync.dma_start(h8[:, 0, :], xt8[j][:, 0, 0:ch0])
                nc.scalar.dma_start(h8[:, 1, :], xt8[j][:, 1, 0:ch0])
                x8h.append(h8)
            # tails: only needed by the chunk>=1 chains, which start after
            # chunk 0 (~57us) - all tails are resident by ~46us. Issued in
            # late-phase consumption order (DR pairs first, then fp16 k).
            x8t = []
            for j in range(NPAIR):
                t8 = xp.tile([128, 2, tail], f8, name=f"x8t{j}", tag=f"x8t{j}")
                nc.sync.dma_start(t8[:, 0, :], xt8[j][:, 0, ch0:n_tok])
                nc.scalar.dma_start(t8[:, 1, :], xt8[j][:, 1, ch0:n_tok])
                x8t.append(t8)
            xtl = []
            for k in range(K16):
                t = xp.tile([128, tail], f16, name=f"xt{k}", tag=f"xt{k}")
                rings[k % 2].dma_start(t[:], xt[k][:, ch0:n_tok])
                xtl.append(t)

            def x_slice(k, s0, n):
                if s0 == 0:
                    return xh[k][:, 0:n]
                return xtl[k][:, s0 - ch0 : s0 - ch0 + n]

            def x8_slice(j, s0, n):
                if s0 == 0:
                    return x8h[j][:, :, 0:n]
                return x8t[j][:, :, s0 - ch0 : s0 - ch0 + n]

            # PE warm-up: matmuls on a zeroed tile, no DMA dependency. Runs
            # during the DMA ramp (PE would idle anyway) and flips the HAM
            # clock gate to 8/8 before the first real matmul. memset on DVE:
            # it boots by ~4.7us and memsets in ~200ns.
            wz = bp.tile([128, 512], f16, name="wz")
            nc.vector.memset(wz[:], 0.0)
            # psw shares the chain-psum rotation: its slot is recycled by the
            # 8th chunk-0 chain, long after the warm-up finishes. The warm-up
            # bridges PE-boot (~8us) to first-operand-ready: with the pair
            # striped across both rings the first (w0-lo, xh0) lands ~1.5us
            # earlier than single-ring, so only a few zero-MMs are needed -
            # excess warm-ups sit AHEAD of ready real work in the PE queue
            # and delay it (they run at p-state-ramp clock, ~430-790ns each).
            psw = pp.tile([128, 512], f32, name="psw", tag="ps")
            for _ in range(N_WARMUP):
                nc.tensor.matmul(psw[:], wz[:, 0:128], wz[:], start=True, stop=True)

            def unit_mm(ps, e, s0, n, u, start, stop):
                if u[0] == "f":
                    return nc.tensor.matmul(
                        ps[:, 0:n],
                        wk[u[1]][:, e * 128 : (e + 1) * 128],
                        x_slice(u[1], s0, n),
                        start=start,
                        stop=stop,
                    )
                return nc.tensor.matmul(
                    ps[:, 0:n],
                    w8t[u[1]][:, :, e * 128 : (e + 1) * 128],
                    x8_slice(u[1], s0, n),
                    start=start,
                    stop=stop,
                    perf_mode=DR,
                )

            def mm_chain(ps, e, s0, n, us=None):
                us = us if us is not None else units
                first = last = None
                for i, u in enumerate(us):
                    mm = unit_mm(ps, e, s0, n, u, i == 0, i == len(us) - 1)
                    first = first or mm
                    last = mm
                return first, last

            prev_last = None

            def pin(first, reason):
                # keep the PE stream in emission order chain-by-chain: the
                # scheduler otherwise hoists later chains (gated on late
                # arrivals) ahead of ready work and stalls the PE
                if prev_last is not None:
                    tile.add_dep_helper(
                        first.ins, prev_last.ins, sync=False, reason=reason
                    )

            def drain(ps, e, s0, n):
                ot = op_.tile([128, n], f16, name=f"ot{e}_{s0}", tag="ot")
                nc.vector.tensor_scalar_add(ot[:], ps[:, 0:n], bias_s[:, e : e + 1])
                # (vector is not a HWDGE engine - y DMAs must ride sync or
                # scalar; sync measured equal-or-better)
                nc.sync.dma_start(y[e][:, s0 : s0 + n], ot[:])

            # phase A: chunk-0 chains, UNPINNED so the scheduler interleaves
            # them by operand arrival during the DMA ramp. 8 psum banks keep
            # 8 chains in flight (8 x 216ns of PE work per arriving k-pair
            # matches the ~1.7us pair cadence); later e-tiles draft behind
            # the frontier on already-arrived pairs.
            a_lasts = []
            s0a, n0 = CH[0]
            for e in range(ET):
                ps = pp.tile([128, 512], f32, name=f"psa{e}", tag="ps")
                fa, la = mm_chain(ps, e, s0a, n0)
                a_lasts.append(la)
                drain(ps, e, s0a, n0)

            # chunks >= 1: all operands are resident by now; strict emission
            # order keeps the PE stream dense. DR units go FIRST so the
            # chain's stop-MM is a plain fp16 one. (Merging late drains into
            # per-e staging tiles with one y-DMA measured SLOWER - the
            # teardown semaphore storm did not shrink with transfer count.)
            units_l = [("d", j) for j in range(NPAIR)] + [("f", k) for k in range(K16)]
            first_late = True
            for s0, n in CH[1:]:
                for e in range(ET):
                    ps = pp.tile([128, 512], f32, name=f"ps{e}_{s0}", tag="ps")
                    ff, lf = mm_chain(ps, e, s0, n, units_l)
                    if first_late:
                        for la in a_lasts:
                            tile.add_dep_helper(ff.ins, la.ins, sync=False, reason="A->F")
                        first_late = False
                    else:
                        pin(ff, f"chain order c{s0}e{e}")
                    prev_last = lf
                    drain(ps, e, s0, n)

    nc.compile()
    return nc


def _get_program(n_tok: int):
    if n_tok not in _PROGRAM_CACHE:
        _PROGRAM_CACHE[n_tok] = _build_program(n_tok)
    return _PROGRAM_CACHE[n_tok]


def _round_up(v: int, m: int) -> int:
    return -(-v // m) * m


def _q8(a: np.ndarray, scale: float) -> np.ndarray:
    return np.clip(a * scale, -240.0, 240.0).astype(F8)


def kernel(hidden_states, type_ids, W0, b0, W1, b1, _trace=False, _tmpdir=None):
    global LAST_RESULTS

    B, S, D_ = hidden_states.shape
    assert D_ == D
    x = np.ascontiguousarray(np.asarray(hidden_states, dtype=np.float32)).reshape(
        B * S, D
    )
    t = np.asarray(type_ids).reshape(B * S)

    idx = [np.nonzero(t == e)[0] for e in (0, 1)]
    counts = [len(i) for i in idx]
    # tokens per core: 4 cores per expert, token dim rounded to 4 (moving
    # operand - no 128 padding needed). Extremely skewed expert splits fall
    # back to multiple launches of the same program over token slices.
    N_TOK_MAX = 4096
    n_tok = max(64, _round_up(-(-max(counts) // CORES_PER_EXPERT), 4))
    n_tok = min(n_tok, N_TOK_MAX)
    cap = n_tok * CORES_PER_EXPERT
    n_launches = -(-max(counts) // cap)

    nc = _get_program(n_tok)

    wts, wt8s, biases = [], [], []
    for W, b in ((W0, b0), (W1, b1)):
        WT = np.asarray(W, dtype=np.float32).T  # [d, e]
        wts.append(
            np.ascontiguousarray((WT[:KSPLIT] * SCALE).astype(np.float16)).reshape(
                K16, 128, D
            )
        )
        # pair j, plane i, partition p  <->  contraction row KSPLIT+256j+128i+p
        wt8s.append(
            np.ascontiguousarray(
                _q8(WT[KSPLIT:], SW).reshape(NPAIR, 2, 128, D).transpose(0, 2, 1, 3)
            )
        )
        biases.append(
            np.ascontiguousarray(
                (np.asarray(b, dtype=np.float32) * SCALE).reshape(ET, 128).T
            )
        )

    gathered = [x[idx[e]] for e in (0, 1)]  # [count_e, D] fp32

    out = np.empty((B * S, D), dtype=np.float32)
    parts = [[], []]
    for li in range(n_launches):
        in_maps = []
        for e in (0, 1):
            g = gathered[e][li * cap : (li + 1) * cap]
            if g.shape[0] < cap:
                g = np.concatenate(
                    [g, np.zeros((cap - g.shape[0], D), np.float32)], axis=0
                )
            for c in range(CORES_PER_EXPERT):
                chunk = g[c * n_tok : (c + 1) * n_tok]  # [n_tok, D] fp32
                ct = chunk.T  # [D, n_tok]
                xt_c = np.ascontiguousarray(ct[:KSPLIT].astype(np.float16)).reshape(
                    K16, 128, n_tok
                )
                xt8_c = np.ascontiguousarray(
                    _q8(ct[KSPLIT:], SX)
                    .reshape(NPAIR, 2, 128, n_tok)
                    .transpose(0, 2, 1, 3)
                )
                in_maps.append(
                    {
                        "xt": xt_c,
                        "xt8": xt8_c,
                        "wt": wts[e],
                        "wt8": wt8s[e],
                        "biasw": biases[e],
                    }
                )

        res = None
        for attempt in range(3):
            try:
                res = run_bass_kernel_spmd(
                    nc, in_maps, list(range(N_CORES)), trace=_trace, tmpdir=_tmpdir
                )
                break
            except Exception:
                # transient NRT_EXEC_UNIT_UNRECOVERABLE has been observed when
                # a run starts right as a previous process tears the device down
                if attempt == 2:
                    raise
                time.sleep(10)
        LAST_RESULTS = res
        for e in (0, 1):
            parts[e].extend(
                res.results[e * CORES_PER_EXPERT + c]["y"].reshape(D, n_tok).T
                for c in range(CORES_PER_EXPERT)
            )

    inv = np.float32(1.0 / SCALE)
    for e in (0, 1):
        full_e = np.concatenate(parts[e], axis=0)[: counts[e]]
        out[idx[e]] = full_e.astype(np.float32) * inv
    return out.reshape(B, S, D)



# revision 15
# speedup vs baseline: 1.2874x; 1.2874x over previous
# Mixture-of-two-experts (modality-routed) token GEMM on 8 Trainium2 NeuronCores.
#
# v4: weights-stationary. The reference computes BOTH expert GEMMs and selects
# per token; only one GEMM per token is needed. Host partitions tokens by
# type_id (expert-dispatch): cores 0-3 carry expert-0 tokens + W0, cores 4-7
# expert-1 tokens + W1 (weights arrive as data, the per-core program is
# identical). On device each core computes y[e, tok] = W x + b with W tiles
# STATIONARY and the token dim MOVING, so PE cost scales with the actual
# per-core token count (n_tok rounded to 16) instead of 128-padded m-tiles.
#
# Mixed precision: contraction k 0..1279 runs fp16; k 1280..2047 runs as three
# fp8e4m3 DoubleRow pairs (2 k-planes per PE cell, 2 MACs/cycle) - 13 PE
# passes over k instead of 16. SW=45.25 places max|W|*SW at 1.0 so the top
# e4m3 binade is [0.5,1) with step 1/16 (SW=64 wastes half the mantissa:
# max lands mid-binade at 1.41 where the step is 1/8). Host-simulated with
# exact device semantics: rel err 1.8555e-2, inside the 2e-2 gate (the b=2
# config's host sim matched hardware to 2e-6).
# Scales: x8 = x*16, w8 = W*45.25, fp16 W pre-scaled *724 so one PSUM chain
# is consistent at 724*y; the host divides the fp16 output by 724.

import os
import sys
import time

import numpy as np
import ml_dtypes

for _p in ("/opt/trn_rl_repo", "/root/.axon_site/_ro/trn_rl_repo"):
    if os.path.isdir(_p) and _p not in sys.path:
        sys.path.insert(0, _p)

import concourse.bacc as bacc
import concourse.mybir as mybir
import concourse.tile as tile
from concourse.bass_utils import run_bass_kernel_spmd

D = 2048
ET = D // 128  # 16 output-feature tiles
K16 = 10  # fp16 k-tiles (k 0..1279)
NPAIR = 3  # fp8 DoubleRow pairs (k 1280..2047, 256 contraction rows each)
KSPLIT = K16 * 128
SX = 16.0  # fp8 scale on x
SW = 45.25  # fp8 scale on W (max|W|*SW = 1.0: top binade step 1/16)
SCALE = SX * SW  # PSUM carries SCALE*y; host divides it out
N_CORES = 8
CORES_PER_EXPERT = 4
N_WARMUP = 6  # PE warm-up matmuls bridging PE-boot (~7.9us) to first-
# operand-ready (~10.7us). Measured: the first ~3.5us of DMA deliver only
# ~0.4MB no matter how transfers are sized/ordered (startup-limited), so
# the first real MM cannot start before ~10.7us; 6 zero-MMs at p-state-
# ramp clock (788+5*427ns) end right there.
F8 = ml_dtypes.float8_e4m3fn

_PROGRAM_CACHE: dict[int, object] = {}
LAST_RESULTS = None  # BassKernelResults of the most recent launch (for profiling)


def _chunks(n_tok: int):
    """Split the token dim into <=512-wide chunks, 4-aligned boundaries.

    Chunk 0 is full 512 wide when possible: during the DMA ramp the 8
    in-flight chunk-0 chains then expose 8*216ns of PE work per arriving
    (w_k, xh_k) pair, matching the ~1.7us pair cadence."""
    if n_tok <= 512:
        return [(0, n_tok)]
    rest = n_tok - 512
    nch = -(-rest // 512)
    base = rest // nch // 4 * 4
    sizes = [base] * nch
    i = 0
    while 512 + sum(sizes) < n_tok:
        sizes[i] += 4
        i = (i + 1) % nch
    sizes = [512] + sizes
    assert 512 + rest == n_tok and all(s <= 512 for s in sizes)
    out, s0 = [], 0
    for s in sizes:
        out.append((s0, s))
        s0 += s
    return out


def _build_program(n_tok: int):
    """One NeuronCore program: y[e, tok] = SCALE * (W @ x + bias), fp16 out."""
    assert n_tok % 4 == 0
    f16 = mybir.dt.float16
    f32 = mybir.dt.float32
    f8 = mybir.dt.float8e4
    DR = mybir.MatmulPerfMode.DoubleRow

    nc = bacc.Bacc("TRN2", target_bir_lowering=False, debug=False, num_devices=N_CORES)
    xt = nc.dram_tensor("xt", [K16, 128, n_tok], f16, kind="ExternalInput").ap()
    xt8 = nc.dram_tensor("xt8", [NPAIR, 128, 2, n_tok], f8, kind="ExternalInput").ap()
    wt = nc.dram_tensor("wt", [K16, 128, D], f16, kind="ExternalInput").ap()
    wt8 = nc.dram_tensor("wt8", [NPAIR, 128, 2, D], f8, kind="ExternalInput").ap()
    biasw = nc.dram_tensor("biasw", [128, ET], f32, kind="ExternalInput").ap()
    y = nc.dram_tensor("y", [ET, 128, n_tok], f16, kind="ExternalOutput").ap()

    CH = _chunks(n_tok)
    ch0 = CH[0][1]  # chunk-0 width: the x "head" loaded before the tails
    tail = n_tok - ch0

    # k-units: ('f', k) = one fp16 k-tile, ('d', j) = one fp8 DoubleRow pair.
    units = [("f", k) for k in range(K16)] + [("d", j) for j in range(NPAIR)]

    with tile.TileContext(nc) as tc:
        with (
            tc.tile_pool(name="wp", bufs=1) as wp,
            tc.tile_pool(name="xp", bufs=1) as xp,
            tc.tile_pool(name="bp", bufs=1) as bp,
            # ot staging: enough bufs that DVE drains never wait on y-DMA
            # completion (the y DMAs queue up behind the input loads on the
            # sync HWDGE ring until ~50us - with few bufs the DVE stalls on
            # slot reuse, the PE runs dry behind it, and HAM re-throttles)
            tc.tile_pool(name="op", bufs=14) as op_,
            tc.tile_pool(name="pp", bufs=8, space="PSUM") as pp,
        ):
            # (w_k, x-head_k) pairs in unit order across BOTH HWDGE rings
            # (sync + scalar). While both rings are backlogged they share
            # the ~360GB/s HBM pool, so only SMOOTHNESS differs from one
            # ring: each w tile's lo/hi halves ride opposite rings (256KB,
            # 2048B rows) and the small heads alternate, so pairs complete
            # in lockstep ~640KB-of-total-traffic apart - no 2-pair bursts
            # (v4) to overflow the 8-chain PSUM buffer. CAUTION: transfers
            # with <1024B rows run at half DMA efficiency (v5 regression:
            # 512B-row xh halves dropped aggregate BW from 360 to 206GB/s);
            # every transfer here keeps rows >= 1024B.
            rings = (nc.sync, nc.scalar)
            bias_s = bp.tile([128, ET], f32, name="bias_s")
            wk, xh = [], []
            for k in range(K16):
                ws = wp.tile([128, D], f16, name=f"w{k}", tag=f"w{k}")
                h = xp.tile([128, ch0], f16, name=f"xh{k}", tag=f"xh{k}")
                nc.sync.dma_start(ws[:, 0 : D // 2], wt[k][:, 0 : D // 2])
                if k == 0:
                    # xh0 leads the scalar ring: the first chain needs only
                    # w0-lo (ring0 head) + xh0 (ring1 head)
                    nc.scalar.dma_start(h[:], xt[k][:, 0:ch0])
                    nc.scalar.dma_start(ws[:, D // 2 : D], wt[k][:, D // 2 : D])
                    nc.scalar.dma_start(bias_s[:], biasw[:])
                else:
                    nc.scalar.dma_start(ws[:, D // 2 : D], wt[k][:, D // 2 : D])
                    rings[k % 2].dma_start(h[:], xt[k][:, 0:ch0])
                wk.append(ws)
                xh.append(h)
            w8t, x8h = [], []
            for j in range(NPAIR):
                w8 = wp.tile([128, 2, D], f8, name=f"w8_{j}", tag=f"w8_{j}")
                nc.sync.dma_start(w8[:, 0, :], wt8[j][:, 0, :])
                nc.scalar.dma_start(w8[:, 1, :], wt8[j][:, 1, :])
                w8t.append(w8)
                h8 = xp.tile([128, 2, ch0], f8, name=f"x8h{j}", tag=f"x8h{j}")
                rings[j % 2].dma_start(h8[:], xt8[j][:, :, 0:ch0])
                x8h.append(h8)
            # tails: only needed by the chunk>=1 chains, which start after
            # chunk 0 - all tails are resident well before that. Issued in
            # late-phase consumption order (DR pairs first, then fp16 k).
            x8t = []
            for j in range(NPAIR):
                t8 = xp.tile([128, 2, tail], f8, name=f"x8t{j}", tag=f"x8t{j}")
                rings[j % 2].dma_start(t8[:], xt8[j][:, :, ch0:n_tok])
                x8t.append(t8)
            xtl = []
            for k in range(K16):
                t = xp.tile([128, tail], f16, name=f"xt{k}", tag=f"xt{k}")
                rings[k % 2].dma_start(t[:], xt[k][:, ch0:n_tok])
                xtl.append(t)

            def x_slice(k, s0, n):
                if s0 == 0:
                    return xh[k][:, 0:n]
                return xtl[k][:, s0 - ch0 : s0 - ch0 + n]

            def x8_slice(j, s0, n):
                if s0 == 0:
                    return x8h[j][:, :, 0:n]
                return x8t[j][:, :, s0 - ch0 : s0 - ch0 + n]

            # PE warm-up: matmuls on a zeroed tile, no DMA dependency. Runs
            # during the DMA ramp (PE would idle anyway) and flips the HAM
            # clock gate to 8/8 before the first real matmul. memset on DVE:
            # it boots by ~4.7us and memsets in ~200ns.
            wz = bp.tile([128, 512], f16, name="wz")
            nc.vector.memset(wz[:], 0.0)
            # psw shares the chain-psum rotation: its slot is recycled by the
            # 8th chunk-0 chain, long after the warm-up finishes. The warm-up
            # bridges PE-boot (~8us) to first-operand-ready: with the pair
            # striped across both rings the first (w0-lo, xh0) lands ~1.5us
            # earlier than single-ring, so only a few zero-MMs are needed -
            # excess warm-ups sit AHEAD of ready real work in the PE queue
            # and delay it (they run at p-state-ramp clock, ~430-790ns each).
            psw = pp.tile([128, 512], f32, name="psw", tag="ps")
            for _ in range(N_WARMUP):
                nc.tensor.matmul(psw[:], wz[:, 0:128], wz[:], start=True, stop=True)

            def unit_mm(ps, e, s0, n, u, start, stop):
                if u[0] == "f":
                    return nc.tensor.matmul(
                        ps[:, 0:n],
                        wk[u[1]][:, e * 128 : (e + 1) * 128],
                        x_slice(u[1], s0, n),
                        start=start,
                        stop=stop,
                    )
                return nc.tensor.matmul(
                    ps[:, 0:n],
                    w8t[u[1]][:, :, e * 128 : (e + 1) * 128],
                    x8_slice(u[1], s0, n),
                    start=start,
                    stop=stop,
                    perf_mode=DR,
                )

            def mm_chain(ps, e, s0, n, us=None):
                us = us if us is not None else units
                first = last = None
                for i, u in enumerate(us):
                    mm = unit_mm(ps, e, s0, n, u, i == 0, i == len(us) - 1)
                    first = first or mm
                    last = mm
                return first, last

            prev_last = None

            def pin(first, reason):
                # keep the PE stream in emission order chain-by-chain: the
                # scheduler otherwise hoists later chains (gated on late
                # arrivals) ahead of ready work and stalls the PE
                if prev_last is not None:
                    tile.add_dep_helper(
                        first.ins, prev_last.ins, sync=False, reason=reason
                    )

            def drain(ps, e, s0, n):
                ot = op_.tile([128, n], f16, name=f"ot{e}_{s0}", tag="ot")
                nc.vector.tensor_scalar_add(ot[:], ps[:, 0:n], bias_s[:, e : e + 1])
                # (vector is not a HWDGE engine - y DMAs must ride sync or
                # scalar; alternate rings so writebacks never pile up behind
                # one ring's input tail and the final completion-wait covers
                # half as much queued traffic)
                rings[e % 2].dma_start(y[e][:, s0 : s0 + n], ot[:])

            # phase A: chunk-0 chains, UNPINNED so the scheduler interleaves
            # them by operand arrival during the DMA ramp. 8 psum banks keep
            # 8 chains in flight (8 x 216ns of PE work per arriving k-pair
            # matches the ~1.7us pair cadence); later e-tiles draft behind
            # the frontier on already-arrived pairs.
            a_lasts = []
            s0a, n0 = CH[0]
            for e in range(ET):
                ps = pp.tile([128, 512], f32, name=f"psa{e}", tag="ps")
                fa, la = mm_chain(ps, e, s0a, n0)
                a_lasts.append(la)
                drain(ps, e, s0a, n0)

            # chunks >= 1: all operands are resident by now; strict emission
            # order keeps the PE stream dense. DR units go FIRST so the
            # chain's stop-MM is a plain fp16 one. (Merging late drains into
            # per-e staging tiles with one y-DMA measured SLOWER - the
            # teardown semaphore storm did not shrink with transfer count.)
            units_l = [("d", j) for j in range(NPAIR)] + [("f", k) for k in range(K16)]
            first_late = True
            for s0, n in CH[1:]:
                for e in range(ET):
                    ps = pp.tile([128, 512], f32, name=f"ps{e}_{s0}", tag="ps")
                    ff, lf = mm_chain(ps, e, s0, n, units_l)
                    if first_late:
                        for la in a_lasts:
                            tile.add_dep_helper(ff.ins, la.ins, sync=False, reason="A->F")
                        first_late = False
                    else:
                        pin(ff, f"chain order c{s0}e{e}")
                    prev_last = lf
                    drain(ps, e, s0, n)

    nc.compile()
    return nc


def _get_program(n_tok: int):
    if n_tok not in _PROGRAM_CACHE:
        _PROGRAM_CACHE[n_tok] = _build_program(n_tok)
    return _PROGRAM_CACHE[n_tok]


def _round_up(v: int, m: int) -> int:
    return -(-v // m) * m


def _q8(a: np.ndarray, scale: float) -> np.ndarray:
    return np.clip(a * scale, -240.0, 240.0).astype(F8)


def kernel(hidden_states, type_ids, W0, b0, W1, b1, _trace=False, _tmpdir=None):
    global LAST_RESULTS

    B, S, D_ = hidden_states.shape
    assert D_ == D
    x = np.ascontiguousarray(np.asarray(hidden_states, dtype=np.float32)).reshape(
        B * S, D
    )
    t = np.asarray(type_ids).reshape(B * S)

    idx = [np.nonzero(t == e)[0] for e in (0, 1)]
    counts = [len(i) for i in idx]
    # tokens per core: 4 cores per expert, token dim rounded to 4 (moving
    # operand - no 128 padding needed). Extremely skewed expert splits fall
    # back to multiple launches of the same program over token slices.
    N_TOK_MAX = 4096
    n_tok = max(64, _round_up(-(-max(counts) // CORES_PER_EXPERT), 4))
    n_tok = min(n_tok, N_TOK_MAX)
    cap = n_tok * CORES_PER_EXPERT
    n_launches = -(-max(counts) // cap)

    nc = _get_program(n_tok)

    wts, wt8s, biases = [], [], []
    for W, b in ((W0, b0), (W1, b1)):
        WT = np.asarray(W, dtype=np.float32).T  # [d, e]
        wts.append(
            np.ascontiguousarray((WT[:KSPLIT] * SCALE).astype(np.float16)).reshape(
                K16, 128, D
            )
        )
        # pair j, plane i, partition p  <->  contraction row KSPLIT+256j+128i+p
        wt8s.append(
            np.ascontiguousarray(
                _q8(WT[KSPLIT:], SW).reshape(NPAIR, 2, 128, D).transpose(0, 2, 1, 3)
            )
        )
        biases.append(
            np.ascontiguousarray(
                (np.asarray(b, dtype=np.float32) * SCALE).reshape(ET, 128).T
            )
        )

    gathered = [x[idx[e]] for e in (0, 1)]  # [count_e, D] fp32

    out = np.empty((B * S, D), dtype=np.float32)
    parts = [[], []]
    for li in range(n_launches):
        in_maps = []
        for e in (0, 1):
            g = gathered[e][li * cap : (li + 1) * cap]
            if g.shape[0] < cap:
                g = np.concatenate(
                    [g, np.zeros((cap - g.shape[0], D), np.float32)], axis=0
                )
            for c in range(CORES_PER_EXPERT):
                chunk = g[c * n_tok : (c + 1) * n_tok]  # [n_tok, D] fp32
                ct = chunk.T  # [D, n_tok]
                xt_c = np.ascontiguousarray(ct[:KSPLIT].astype(np.float16)).reshape(
                    K16, 128, n_tok
                )
                xt8_c = np.ascontiguousarray(
                    _q8(ct[KSPLIT:], SX)
                    .reshape(NPAIR, 2, 128, n_tok)
                    .transpose(0, 2, 1, 3)
                )
                in_maps.append(
                    {
                        "xt": xt_c,
                        "xt8": xt8_c,
                        "wt": wts[e],
                        "wt8": wt8s[e],
                        "biasw": biases[e],
                    }
                )

        res = None
        for attempt in range(3):
            try:
                res = run_bass_kernel_spmd(
                    nc, in_maps, list(range(N_CORES)), trace=_trace, tmpdir=_tmpdir
                )
                break
            except Exception:
                # transient NRT_EXEC_UNIT_UNRECOVERABLE has been observed when
                # a run starts right as a previous process tears the device down
                if attempt == 2:
                    raise
                time.sleep(10)
        LAST_RESULTS = res
        for e in (0, 1):
            parts[e].extend(
                res.results[e * CORES_PER_EXPERT + c]["y"].reshape(D, n_tok).T
                for c in range(CORES_PER_EXPERT)
            )

    inv = np.float32(1.0 / SCALE)
    for e in (0, 1):
        full_e = np.concatenate(parts[e], axis=0)[: counts[e]]
        out[idx[e]] = full_e.astype(np.float32) * inv
    return out.reshape(B, S, D)



# revision 16
# speedup vs baseline: 1.3597x; 1.0562x over previous
# Mixture-of-two-experts (modality-routed) token GEMM on 8 Trainium2 NeuronCores.
#
# v4: weights-stationary. The reference computes BOTH expert GEMMs and selects
# per token; only one GEMM per token is needed. Host partitions tokens by
# type_id (expert-dispatch): cores 0-3 carry expert-0 tokens + W0, cores 4-7
# expert-1 tokens + W1 (weights arrive as data, the per-core program is
# identical). On device each core computes y[e, tok] = W x + b with W tiles
# STATIONARY and the token dim MOVING, so PE cost scales with the actual
# per-core token count (n_tok rounded to 16) instead of 128-padded m-tiles.
#
# Mixed precision: contraction k 0..1279 runs fp16; k 1280..2047 runs as three
# fp8e4m3 DoubleRow pairs (2 k-planes per PE cell, 2 MACs/cycle) - 13 PE
# passes over k instead of 16. SW=45.25 places max|W|*SW at 1.0 so the top
# e4m3 binade is [0.5,1) with step 1/16 (SW=64 wastes half the mantissa:
# max lands mid-binade at 1.41 where the step is 1/8). Host-simulated with
# exact device semantics: rel err 1.8555e-2, inside the 2e-2 gate (the b=2
# config's host sim matched hardware to 2e-6).
# Scales: x8 = x*16, w8 = W*45.25, fp16 W pre-scaled *724 so one PSUM chain
# is consistent at 724*y; the host divides the fp16 output by 724.

import os
import sys
import time

import numpy as np
import ml_dtypes

for _p in ("/opt/trn_rl_repo", "/root/.axon_site/_ro/trn_rl_repo"):
    if os.path.isdir(_p) and _p not in sys.path:
        sys.path.insert(0, _p)

import concourse.bacc as bacc
import concourse.mybir as mybir
import concourse.tile as tile
from concourse.bass_utils import run_bass_kernel_spmd

D = 2048
ET = D // 128  # 16 output-feature tiles
K16 = 10  # fp16 k-tiles (k 0..1279)
NPAIR = 3  # fp8 DoubleRow pairs (k 1280..2047, 256 contraction rows each)
KSPLIT = K16 * 128
SX = 16.0  # fp8 scale on x
SW = 45.25  # fp8 scale on W (max|W|*SW = 1.0: top binade step 1/16)
SCALE = SX * SW  # PSUM carries SCALE*y; host divides it out
N_CORES = 8
CORES_PER_EXPERT = 4
N_WARMUP = 6  # PE warm-up matmuls bridging PE-boot (~7.9us) to first-
# operand-ready (~10.7us). Measured: the first ~3.5us of DMA deliver only
# ~0.4MB no matter how transfers are sized/ordered (startup-limited), so
# the first real MM cannot start before ~10.7us; 6 zero-MMs at p-state-
# ramp clock (788+5*427ns) end right there.
F8 = ml_dtypes.float8_e4m3fn

_PROGRAM_CACHE: dict[int, object] = {}
LAST_RESULTS = None  # BassKernelResults of the most recent launch (for profiling)


def _chunks(n_tok: int):
    """Split the token dim into <=512-wide chunks, 4-aligned boundaries.

    Chunk 0 is full 512 wide when possible: during the DMA ramp the 8
    in-flight chunk-0 chains then expose 8*216ns of PE work per arriving
    (w_k, xh_k) pair, matching the ~1.7us pair cadence."""
    if n_tok <= 512:
        return [(0, n_tok)]
    rest = n_tok - 512
    nch = -(-rest // 512)
    base = rest // nch // 4 * 4
    sizes = [base] * nch
    i = 0
    while 512 + sum(sizes) < n_tok:
        sizes[i] += 4
        i = (i + 1) % nch
    sizes = [512] + sizes
    assert 512 + rest == n_tok and all(s <= 512 for s in sizes)
    out, s0 = [], 0
    for s in sizes:
        out.append((s0, s))
        s0 += s
    return out


def _build_program(n_tok: int):
    """One NeuronCore program: y[e, tok] = SCALE * (W @ x + bias), fp16 out."""
    assert n_tok % 4 == 0
    f16 = mybir.dt.float16
    f32 = mybir.dt.float32
    f8 = mybir.dt.float8e4
    DR = mybir.MatmulPerfMode.DoubleRow

    nc = bacc.Bacc("TRN2", target_bir_lowering=False, debug=False, num_devices=N_CORES)
    xt = nc.dram_tensor("xt", [K16, 128, n_tok], f16, kind="ExternalInput").ap()
    xt8 = nc.dram_tensor("xt8", [NPAIR, 128, 2, n_tok], f8, kind="ExternalInput").ap()
    wt = nc.dram_tensor("wt", [K16, 128, D], f16, kind="ExternalInput").ap()
    wt8 = nc.dram_tensor("wt8", [NPAIR, 128, 2, D], f8, kind="ExternalInput").ap()
    biasw = nc.dram_tensor("biasw", [128, ET], f32, kind="ExternalInput").ap()
    y = nc.dram_tensor("y", [ET, 128, n_tok], f16, kind="ExternalOutput").ap()

    CH = _chunks(n_tok)
    ch0 = CH[0][1]  # chunk-0 width: the x "head" loaded before the tails
    tail = n_tok - ch0

    # k-units: ('f', k) = one fp16 k-tile, ('d', j) = one fp8 DoubleRow pair.
    units = [("f", k) for k in range(K16)] + [("d", j) for j in range(NPAIR)]

    with tile.TileContext(nc) as tc:
        with (
            tc.tile_pool(name="wp", bufs=1) as wp,
            tc.tile_pool(name="xp", bufs=1) as xp,
            tc.tile_pool(name="bp", bufs=1) as bp,
            # ot staging: enough bufs that DVE drains never wait on y-DMA
            # completion (the y DMAs queue up behind the input loads on the
            # sync HWDGE ring until ~50us - with few bufs the DVE stalls on
            # slot reuse, the PE runs dry behind it, and HAM re-throttles)
            tc.tile_pool(name="op", bufs=14) as op_,
            tc.tile_pool(name="pp", bufs=8, space="PSUM") as pp,
        ):
            # (w_k, x-head_k) pairs in unit order on ONE ring (sync): a
            # single priority-ordered FIFO gives each pair the FULL HBM
            # bandwidth in turn. Striping pairs across both HWDGE rings was
            # measured SLOWER every way (v4 whole-tile alternation: bursty
            # arrival, +5.7us of PE gaps; v5/v6 half-striping: pair-ready =
            # max of two jittery half-rate streams). Issue instructions cost
            # ~600-740ns on the engine with a 4-deep in-flight credit
            # window, so the head sequence also wants FEW, BIG transfers:
            # w tiles go whole (512KB) except w0, whose lo half leads so
            # chains e0-e7 can start the moment (w0-lo, xh0) lands. Rows
            # must stay >=1024B: 512B-row transfers halve DMA efficiency
            # (v5: 360 -> 206GB/s aggregate).
            rings = (nc.sync, nc.scalar)
            bias_s = bp.tile([128, ET], f32, name="bias_s")
            wk, xh = [], []
            for k in range(K16):
                ws = wp.tile([128, D], f16, name=f"w{k}", tag=f"w{k}")
                h = xp.tile([128, ch0], f16, name=f"xh{k}", tag=f"xh{k}")
                if k == 0:
                    nc.sync.dma_start(ws[:, 0 : D // 2], wt[k][:, 0 : D // 2])
                    nc.sync.dma_start(h[:], xt[k][:, 0:ch0])
                    nc.sync.dma_start(ws[:, D // 2 : D], wt[k][:, D // 2 : D])
                    nc.sync.dma_start(bias_s[:], biasw[:])
                else:
                    nc.sync.dma_start(ws[:], wt[k])
                    nc.sync.dma_start(h[:], xt[k][:, 0:ch0])
                wk.append(ws)
                xh.append(h)
            w8t, x8h = [], []
            for j in range(NPAIR):
                w8 = wp.tile([128, 2, D], f8, name=f"w8_{j}", tag=f"w8_{j}")
                nc.sync.dma_start(w8[:], wt8[j])
                w8t.append(w8)
                h8 = xp.tile([128, 2, ch0], f8, name=f"x8h{j}", tag=f"x8h{j}")
                nc.sync.dma_start(h8[:], xt8[j][:, :, 0:ch0])
                x8h.append(h8)
            # tails: only needed by the chunk>=1 chains, which start after
            # chunk 0 - all tails are resident well before that. Issued in
            # late-phase consumption order (DR pairs first, then fp16 k).
            x8t = []
            for j in range(NPAIR):
                t8 = xp.tile([128, 2, tail], f8, name=f"x8t{j}", tag=f"x8t{j}")
                nc.sync.dma_start(t8[:], xt8[j][:, :, ch0:n_tok])
                x8t.append(t8)
            xtl = []
            for k in range(K16):
                t = xp.tile([128, tail], f16, name=f"xt{k}", tag=f"xt{k}")
                nc.sync.dma_start(t[:], xt[k][:, ch0:n_tok])
                xtl.append(t)

            def x_slice(k, s0, n):
                if s0 == 0:
                    return xh[k][:, 0:n]
                return xtl[k][:, s0 - ch0 : s0 - ch0 + n]

            def x8_slice(j, s0, n):
                if s0 == 0:
                    return x8h[j][:, :, 0:n]
                return x8t[j][:, :, s0 - ch0 : s0 - ch0 + n]

            # PE warm-up: matmuls on a zeroed tile, no DMA dependency. Runs
            # during the DMA ramp (PE would idle anyway) and flips the HAM
            # clock gate to 8/8 before the first real matmul. memset on DVE:
            # it boots by ~4.7us and memsets in ~200ns.
            wz = bp.tile([128, 512], f16, name="wz")
            nc.vector.memset(wz[:], 0.0)
            # psw shares the chain-psum rotation: its slot is recycled by the
            # 8th chunk-0 chain, long after the warm-up finishes. The warm-up
            # bridges PE-boot (~8us) to first-operand-ready: with the pair
            # striped across both rings the first (w0-lo, xh0) lands ~1.5us
            # earlier than single-ring, so only a few zero-MMs are needed -
            # excess warm-ups sit AHEAD of ready real work in the PE queue
            # and delay it (they run at p-state-ramp clock, ~430-790ns each).
            psw = pp.tile([128, 512], f32, name="psw", tag="ps")
            for _ in range(N_WARMUP):
                nc.tensor.matmul(psw[:], wz[:, 0:128], wz[:], start=True, stop=True)

            def unit_mm(ps, e, s0, n, u, start, stop):
                if u[0] == "f":
                    return nc.tensor.matmul(
                        ps[:, 0:n],
                        wk[u[1]][:, e * 128 : (e + 1) * 128],
                        x_slice(u[1], s0, n),
                        start=start,
                        stop=stop,
                    )
                return nc.tensor.matmul(
                    ps[:, 0:n],
                    w8t[u[1]][:, :, e * 128 : (e + 1) * 128],
                    x8_slice(u[1], s0, n),
                    start=start,
                    stop=stop,
                    perf_mode=DR,
                )

            def mm_chain(ps, e, s0, n, us=None):
                us = us if us is not None else units
                first = last = None
                for i, u in enumerate(us):
                    mm = unit_mm(ps, e, s0, n, u, i == 0, i == len(us) - 1)
                    first = first or mm
                    last = mm
                return first, last

            prev_last = None

            def pin(first, reason):
                # keep the PE stream in emission order chain-by-chain: the
                # scheduler otherwise hoists later chains (gated on late
                # arrivals) ahead of ready work and stalls the PE
                if prev_last is not None:
                    tile.add_dep_helper(
                        first.ins, prev_last.ins, sync=False, reason=reason
                    )

            def drain(ps, e, s0, n):
                ot = op_.tile([128, n], f16, name=f"ot{e}_{s0}", tag="ot")
                nc.vector.tensor_scalar_add(ot[:], ps[:, 0:n], bias_s[:, e : e + 1])
                # (vector is not a HWDGE engine - y DMAs must ride sync or
                # scalar; alternate rings so writebacks never pile up behind
                # one ring's input tail and the final completion-wait covers
                # half as much queued traffic)
                rings[e % 2].dma_start(y[e][:, s0 : s0 + n], ot[:])

            # phase A: chunk-0 chains, UNPINNED so the scheduler interleaves
            # them by operand arrival during the DMA ramp. 8 psum banks keep
            # 8 chains in flight (8 x 216ns of PE work per arriving k-pair
            # matches the ~1.7us pair cadence); later e-tiles draft behind
            # the frontier on already-arrived pairs.
            a_lasts = []
            s0a, n0 = CH[0]
            for e in range(ET):
                ps = pp.tile([128, 512], f32, name=f"psa{e}", tag="ps")
                fa, la = mm_chain(ps, e, s0a, n0)
                a_lasts.append(la)
                drain(ps, e, s0a, n0)

            # chunks >= 1: all operands are resident by now; strict emission
            # order keeps the PE stream dense. DR units go FIRST so the
            # chain's stop-MM is a plain fp16 one. (Merging late drains into
            # per-e staging tiles with one y-DMA measured SLOWER - the
            # teardown semaphore storm did not shrink with transfer count.)
            units_l = [("d", j) for j in range(NPAIR)] + [("f", k) for k in range(K16)]
            first_late = True
            for s0, n in CH[1:]:
                for e in range(ET):
                    ps = pp.tile([128, 512], f32, name=f"ps{e}_{s0}", tag="ps")
                    ff, lf = mm_chain(ps, e, s0, n, units_l)
                    if first_late:
                        for la in a_lasts:
                            tile.add_dep_helper(ff.ins, la.ins, sync=False, reason="A->F")
                        first_late = False
                    else:
                        pin(ff, f"chain order c{s0}e{e}")
                    prev_last = lf
                    drain(ps, e, s0, n)

    nc.compile()
    return nc


def _get_program(n_tok: int):
    if n_tok not in _PROGRAM_CACHE:
        _PROGRAM_CACHE[n_tok] = _build_program(n_tok)
    return _PROGRAM_CACHE[n_tok]


def _round_up(v: int, m: int) -> int:
    return -(-v // m) * m


def _q8(a: np.ndarray, scale: float) -> np.ndarray:
    return np.clip(a * scale, -240.0, 240.0).astype(F8)


def kernel(hidden_states, type_ids, W0, b0, W1, b1, _trace=False, _tmpdir=None):
    global LAST_RESULTS

    B, S, D_ = hidden_states.shape
    assert D_ == D
    x = np.ascontiguousarray(np.asarray(hidden_states, dtype=np.float32)).reshape(
        B * S, D
    )
    t = np.asarray(type_ids).reshape(B * S)

    idx = [np.nonzero(t == e)[0] for e in (0, 1)]
    counts = [len(i) for i in idx]
    # tokens per core: 4 cores per expert, token dim rounded to 4 (moving
    # operand - no 128 padding needed). Extremely skewed expert splits fall
    # back to multiple launches of the same program over token slices.
    N_TOK_MAX = 4096
    n_tok = max(64, _round_up(-(-max(counts) // CORES_PER_EXPERT), 4))
    n_tok = min(n_tok, N_TOK_MAX)
    cap = n_tok * CORES_PER_EXPERT
    n_launches = -(-max(counts) // cap)

    nc = _get_program(n_tok)

    wts, wt8s, biases = [], [], []
    for W, b in ((W0, b0), (W1, b1)):
        WT = np.asarray(W, dtype=np.float32).T  # [d, e]
        wts.append(
            np.ascontiguousarray((WT[:KSPLIT] * SCALE).astype(np.float16)).reshape(
                K16, 128, D
            )
        )
        # pair j, plane i, partition p  <->  contraction row KSPLIT+256j+128i+p
        wt8s.append(
            np.ascontiguousarray(
                _q8(WT[KSPLIT:], SW).reshape(NPAIR, 2, 128, D).transpose(0, 2, 1, 3)
            )
        )
        biases.append(
            np.ascontiguousarray(
                (np.asarray(b, dtype=np.float32) * SCALE).reshape(ET, 128).T
            )
        )

    gathered = [x[idx[e]] for e in (0, 1)]  # [count_e, D] fp32

    out = np.empty((B * S, D), dtype=np.float32)
    parts = [[], []]
    for li in range(n_launches):
        in_maps = []
        for e in (0, 1):
            g = gathered[e][li * cap : (li + 1) * cap]
            if g.shape[0] < cap:
                g = np.concatenate(
                    [g, np.zeros((cap - g.shape[0], D), np.float32)], axis=0
                )
            for c in range(CORES_PER_EXPERT):
                chunk = g[c * n_tok : (c + 1) * n_tok]  # [n_tok, D] fp32
                ct = chunk.T  # [D, n_tok]
                xt_c = np.ascontiguousarray(ct[:KSPLIT].astype(np.float16)).reshape(
                    K16, 128, n_tok
                )
                xt8_c = np.ascontiguousarray(
                    _q8(ct[KSPLIT:], SX)
                    .reshape(NPAIR, 2, 128, n_tok)
                    .transpose(0, 2, 1, 3)
                )
                in_maps.append(
                    {
                        "xt": xt_c,
                        "xt8": xt8_c,
                        "wt": wts[e],
                        "wt8": wt8s[e],
                        "biasw": biases[e],
                    }
                )

        res = None
        for attempt in range(3):
            try:
                res = run_bass_kernel_spmd(
                    nc, in_maps, list(range(N_CORES)), trace=_trace, tmpdir=_tmpdir
                )
                break
            except Exception:
                # transient NRT_EXEC_UNIT_UNRECOVERABLE has been observed when
                # a run starts right as a previous process tears the device down
                if attempt == 2:
                    raise
                time.sleep(10)
        LAST_RESULTS = res
        for e in (0, 1):
            parts[e].extend(
                res.results[e * CORES_PER_EXPERT + c]["y"].reshape(D, n_tok).T
                for c in range(CORES_PER_EXPERT)
            )

    inv = np.float32(1.0 / SCALE)
    for e in (0, 1):
        full_e = np.concatenate(parts[e], axis=0)[: counts[e]]
        out[idx[e]] = full_e.astype(np.float32) * inv
    return out.reshape(B, S, D)

